# revision 1
# baseline (speedup 1.0000x reference)
"""NonLocalAttention Trainium2 kernel.

Math per batch b (reference):
  q/k/v = conv1x1(x, w*, b*)            # [CI, N], N = H*W = 4096, CI = 128
  attn  = softmax(q^T k, axis=-1)       # [N, N]
  o     = v @ attn^T                    # [CI, N]
  out   = gamma * (wo @ o + bo) + x     # [C, N]

Distribution: data-parallel over batch, one batch per NeuronCore (B = 8 = n_cores).

Per-core algorithm (all big matmuls in bf16, fp32 PSUM accumulation):
  - Q, K        = wT.T @ x  (+bias)              [CI=128 part, N free]
  - V^T chunks  = x_chunk.T @ wvT                [128 j-rows, CI]  (bias folded out, see below)
  - S^T[j, i]   = K_chunk.T @ Q  computed directly with j on partitions -> no transposes
  - A_u         = exp(S^T) on ScalarE (no max subtraction needed: logits are O(±8), fp32 exp safe)
  - O_u[c, i]   = sum_jc VT_chunk.T @ A_u_chunk  (PSUM accumulation over 32 chunks)
  - sums[*, i]  = sum_jc ones.T @ A_u_chunk      (softmax denominators via PE matvec)
  - O = O_u * (1/sums); out = gamma*(woT.T @ O) + gbo + x
  - softmax row-normalization commutes with the V and wo matmuls; the V-bias term
    contributes bv[c] * sum_j A[i,j]/sums[i] = bv[c], so it folds into a host-side
    constant gbo = gamma*(wo@bv + bo).
"""

import numpy as np
import ml_dtypes

B, C = 8, 256
HH, WW = 64, 64
N = HH * WW          # 4096
CI = 128
P = 128
IB = 1024            # i-block (columns of S^T processed per PSUM round)
NIB = N // IB        # 4
NJC = N // P         # 32 j-chunks
FD = 512             # matmul moving-operand free dim (one PSUM bank fp32)
NCORES = 8

_CACHE = {}


def _build(reps=1):
    key = ("nc", reps)
    if key in _CACHE:
        return _CACHE[key]
    from contextlib import ExitStack
    import concourse.bacc as bacc
    import concourse.tile as tile
    from concourse import mybir

    f32 = mybir.dt.float32
    bf16 = mybir.dt.bfloat16
    EXP = mybir.ActivationFunctionType.Exp

    nc = bacc.Bacc("TRN2", target_bir_lowering=False, debug=False, num_devices=NCORES)

    x_f = nc.dram_tensor("x_f", [2, P, N], f32, kind="ExternalInput").ap()
    x_b = nc.dram_tensor("x_b", [2, P, N], bf16, kind="ExternalInput").ap()
    wqT_d = nc.dram_tensor("wqT", [2, P, CI], bf16, kind="ExternalInput").ap()
    wkT_d = nc.dram_tensor("wkT", [2, P, CI], bf16, kind="ExternalInput").ap()
    wvT_d = nc.dram_tensor("wvT", [2, P, CI], bf16, kind="ExternalInput").ap()
    woT_d = nc.dram_tensor("woT", [P, C], bf16, kind="ExternalInput").ap()
    bq_d = nc.dram_tensor("bq", [P, 1], f32, kind="ExternalInput").ap()
    bk_d = nc.dram_tensor("bk", [P, 1], f32, kind="ExternalInput").ap()
    gbo_d = nc.dram_tensor("gbo", [P, 2], f32, kind="ExternalInput").ap()
    gam_d = nc.dram_tensor("gam", [P, 1], f32, kind="ExternalInput").ap()
    out_d = nc.dram_tensor("out", [C, N], f32, kind="ExternalOutput").ap()

    with tile.TileContext(nc) as tc, ExitStack() as ctx:
        sb = ctx.enter_context(tc.tile_pool(name="sb", bufs=1))
        wk_pool = ctx.enter_context(tc.tile_pool(name="wk", bufs=1))
        ps = ctx.enter_context(tc.tile_pool(name="ps", bufs=1, space="PSUM"))

        # ---- persistent SBUF tensors ----
        Xf = [sb.tile([P, N], f32, name=f"Xf{c}") for c in range(2)]
        Xb = [sb.tile([P, N], bf16, name=f"Xb{c}") for c in range(2)]
        Qs = sb.tile([P, N], bf16, name="Qs")
        Ks = sb.tile([P, N], bf16, name="Ks")
        VT = sb.tile([P, N], bf16, name="VT")
        wqT_s = sb.tile([P, C], bf16, name="wqT_s")
        wkT_s = sb.tile([P, C], bf16, name="wkT_s")
        wvT_s = sb.tile([P, C], bf16, name="wvT_s")
        woT_s = sb.tile([P, C], bf16, name="woT_s")
        bq_s = sb.tile([P, 1], f32, name="bq_s")
        bk_s = sb.tile([P, 1], f32, name="bk_s")
        gbo_s = sb.tile([P, 2], f32, name="gbo_s")
        gam_s = sb.tile([P, 1], f32, name="gam_s")
        ones_s = sb.tile([P, P], bf16, name="ones_s")

        # ---- input DMAs ----
        for cc in range(2):
            nc.sync.dma_start(out=wqT_s[:, cc * CI:(cc + 1) * CI], in_=wqT_d[cc])
            nc.sync.dma_start(out=wkT_s[:, cc * CI:(cc + 1) * CI], in_=wkT_d[cc])
            nc.sync.dma_start(out=wvT_s[:, cc * CI:(cc + 1) * CI], in_=wvT_d[cc])
        nc.sync.dma_start(out=woT_s, in_=woT_d)
        nc.sync.dma_start(out=bq_s, in_=bq_d)
        nc.sync.dma_start(out=bk_s, in_=bk_d)
        nc.sync.dma_start(out=gbo_s, in_=gbo_d)
        nc.sync.dma_start(out=gam_s, in_=gam_d)
        nc.vector.memset(ones_s, 1.0)
        for cc in range(2):
            for q in range(4):
                sl = slice(q * 1024, (q + 1) * 1024)
                nc.sync.dma_start(out=Xb[cc][:, sl], in_=x_b[cc, :, sl])
        for cc in range(2):
            for q in range(4):
                sl = slice(q * 1024, (q + 1) * 1024)
                nc.sync.dma_start(out=Xf[cc][:, sl], in_=x_f[cc, :, sl])

        # ---- Q, K projections: [CI, N] bf16, bias added on DVE during PSUM->SBUF ----
        for wname, W_s, b_s, OUT in (("q", wqT_s, bq_s, Qs), ("k", wkT_s, bk_s, Ks)):
            for s8 in range(N // FD):
                sl = slice(s8 * FD, (s8 + 1) * FD)
                pj = ps.tile([P, FD], f32, tag="st", bufs=2, name=f"p{wname}{s8}")
                for cc in range(2):
                    nc.tensor.matmul(
                        pj, lhsT=W_s[:, cc * CI:(cc + 1) * CI], rhs=Xb[cc][:, sl],
                        start=(cc == 0), stop=(cc == 1))
                nc.vector.tensor_scalar_add(out=OUT[:, sl], in0=pj, scalar1=b_s)

        # ---- V^T: chunk jc is [128 rows of n, CI] at VT[:, jc*128:(jc+1)*128] ----
        for jc in range(NJC):
            slj = slice(jc * P, (jc + 1) * P)
            pv = ps.tile([P, P], f32, tag="st", bufs=2, name=f"pv{jc}")
            for cc in range(2):
                nc.tensor.matmul(
                    pv, lhsT=Xb[cc][:, slj], rhs=wvT_s[:, cc * CI:(cc + 1) * CI],
                    start=(cc == 0), stop=(cc == 1))
            nc.vector.tensor_copy(out=VT[:, slj], in_=pv)

        # ---- attention main loop ----
        for _rep in range(reps):
            _main(nc, tc, ps, wk_pool, mybir, f32, bf16, EXP,
                  Xf, Qs, Ks, VT, woT_s, gbo_s, gam_s, ones_s, out_d)

    nc.compile()
    _CACHE[key] = nc
    return nc


def _main(nc, tc, ps, wk_pool, mybir, f32, bf16, EXP,
          Xf, Qs, Ks, VT, woT_s, gbo_s, gam_s, ones_s, out_d):
    if True:
        def do_st(ib, jc):
            """S^T chunk [j=128, i=IB] -> exp -> bf16 SBUF."""
            i0 = ib * IB
            st_ps = ps.tile([P, IB], f32, tag="st", bufs=2, name=f"st{ib}_{jc}")
            for h in range(IB // FD):
                sl = slice(h * FD, (h + 1) * FD)
                nc.tensor.matmul(
                    st_ps[:, sl],
                    lhsT=Ks[:, jc * P:(jc + 1) * P],
                    rhs=Qs[:, i0 + h * FD: i0 + (h + 1) * FD],
                    start=True, stop=True)
            a_sb = wk_pool.tile([P, IB], bf16, tag="a", bufs=4, name=f"a{ib}_{jc}")
            nc.scalar.activation(a_sb, st_ps, EXP)
            return a_sb

        prefetched = []  # next ib's first S^T chunks, emitted before this ib's tail
        for ib in range(NIB):
            i0 = ib * IB
            o_ps = ps.tile([P, IB], f32, tag="o", bufs=1, name=f"o{ib}")
            s_ps = ps.tile([P, IB], f32, tag="sums", bufs=1, name=f"s{ib}")
            pre, prefetched = prefetched, []
            a_cur = pre.pop(0) if pre else do_st(ib, 0)
            for jc in range(NJC):
                if jc + 1 < NJC:
                    a_next = pre.pop(0) if pre else do_st(ib, jc + 1)
                else:
                    a_next = None
                    if ib + 1 < NIB:
                        # keep PE fed through the tail (recip/mul on DVE)
                        prefetched = [do_st(ib + 1, 0), do_st(ib + 1, 1)]
                for h in range(IB // FD):
                    sl = slice(h * FD, (h + 1) * FD)
                    nc.tensor.matmul(
                        o_ps[:, sl], lhsT=VT[:, jc * P:(jc + 1) * P], rhs=a_cur[:, sl],
                        start=(jc == 0), stop=(jc == NJC - 1))
                    nc.tensor.matmul(
                        s_ps[:, sl], lhsT=ones_s, rhs=a_cur[:, sl],
                        start=(jc == 0), stop=(jc == NJC - 1))
                a_cur = a_next

            rec = wk_pool.tile([P, IB], f32, tag="rec", bufs=2, name=f"rec{ib}")
            nc.vector.reciprocal(rec, s_ps)
            onorm = wk_pool.tile([P, IB], bf16, tag="onorm", bufs=2, name=f"on{ib}")
            nc.vector.tensor_mul(onorm, o_ps, rec)

            # ---- output projection + residual for this i-block ----
            for ch in range(2):
                z_ps = ps.tile([P, IB], f32, tag="st", bufs=2, name=f"z{ib}_{ch}")
                for h in range(IB // FD):
                    sl = slice(h * FD, (h + 1) * FD)
                    nc.tensor.matmul(
                        z_ps[:, sl], lhsT=woT_s[:, ch * CI:(ch + 1) * CI],
                        rhs=onorm[:, sl], start=True, stop=True)
                y_sb = wk_pool.tile([P, IB], f32, tag="y", bufs=2, name=f"y{ib}_{ch}")
                # y = gamma*z + gbo[ch]
                nc.vector.tensor_scalar(
                    out=y_sb, in0=z_ps, scalar1=gam_s, scalar2=gbo_s[:, ch:ch + 1],
                    op0=mybir.AluOpType.mult, op1=mybir.AluOpType.add)
                nc.vector.tensor_add(y_sb, y_sb, Xf[ch][:, i0:i0 + IB])
                nc.sync.dma_start(
                    out=out_d[ch * P:(ch + 1) * P, i0:i0 + IB], in_=y_sb)

def _in_maps(x, wq, bq, wk, bk, wv, bv, wo, bo, gamma):
    bf = ml_dtypes.bfloat16
    x = np.asarray(x, np.float32).reshape(B, 2, P, N)
    wq = np.asarray(wq, np.float32)
    wk = np.asarray(wk, np.float32)
    wv = np.asarray(wv, np.float32)
    wo = np.asarray(wo, np.float32)
    bq = np.asarray(bq, np.float32)
    bk = np.asarray(bk, np.float32)
    bv = np.asarray(bv, np.float32)
    bo = np.asarray(bo, np.float32)
    g = float(np.asarray(gamma, np.float32)[0])

    wqT = np.ascontiguousarray(wq.T).reshape(2, P, CI).astype(bf)
    wkT = np.ascontiguousarray(wk.T).reshape(2, P, CI).astype(bf)
    wvT = np.ascontiguousarray(wv.T).reshape(2, P, CI).astype(bf)
    woT = np.ascontiguousarray(wo.T).astype(bf)                     # [CI, C]
    gbo = np.ascontiguousarray((g * (wo @ bv + bo)).reshape(2, P).T).astype(np.float32)
    gam = np.full((P, 1), g, np.float32)
    bq2 = np.ascontiguousarray(bq.reshape(P, 1))
    bk2 = np.ascontiguousarray(bk.reshape(P, 1))

    maps = []
    for b in range(B):
        xb = np.ascontiguousarray(x[b])
        maps.append(dict(
            x_f=xb, x_b=xb.astype(bf), wqT=wqT, wkT=wkT, wvT=wvT, woT=woT,
            bq=bq2, bk=bk2, gbo=gbo, gam=gam))
    return maps


def run(trace=False, **inputs):
    import concourse.bass_utils as bass_utils
    nc = _build()
    maps = _in_maps(**inputs)
    res = bass_utils.run_bass_kernel_spmd(
        nc, maps, core_ids=list(range(NCORES)), trace=trace)
    out = np.stack([r["out"] for r in res.results])
    return out.reshape(B, C, HH, WW).astype(np.float32), res


def kernel(**inputs):
    out, _ = run(trace=False, **inputs)
    return out



# revision 2
# speedup vs baseline: 1.2258x; 1.2258x over previous
"""NonLocalAttention Trainium2 kernel (fp8 DoubleRow + split-engine softmax).

Math per batch b (reference):
  q/k/v = conv1x1(x, w*, b*)            # [CI, N], N = H*W = 4096, CI = 128
  attn  = softmax(q^T k, axis=-1)       # [N, N]
  o     = v @ attn^T                    # [CI, N]
  out   = gamma * (wo @ o + bo) + x     # [C, N]

Distribution: data-parallel over batch, one batch per NeuronCore (B = 8).

Per-core design:
  - All heavy matmuls run in fp8 with MatmulPerfMode.DoubleRow (2 k-subtiles of
    128 contracted per pass at 0.5 cyc/row):
      * Q/K/V projections contract C=256 -> full-rate DoubleRow.
      * S^T = K^T Q contracts CI=128 -> DoubleRow with a zeroed 2nd k-subtile.
      * O = V @ A and sums = 1^T A contract N=4096 -> 16 pair-chunks each.
  - softmax (no max subtraction needed; |logits| <= ~9):
      * A = exp(S^T) stored as fp8e5m2 (max 57344 > e^9; sums computed from the
        SAME quantized A so normalization stays consistent).
      * exp work is split across ScalarE (native Exp activation) and DVE
        (Schraudolph bit trick: e5m2 bits of exp(s) == rint(4/ln2 * s + c),
        computed by tensor_scalar into an int8 alias of the fp8 tile).
  - Softmax row-normalization commutes with the V and wo matmuls; the V-bias
    contributes bv exactly after normalization, so host folds gamma*(wo@bv+bo)
    into the residual tensor xg = x + gamma*(wo@bv + bo), and gamma into
    wog = gamma*wo^T. Final: out = wog^T @ (O/sums) + xg.
"""

import numpy as np
import ml_dtypes

B, C = 8, 256
HH, WW = 64, 64
N = HH * WW          # 4096
CI = 128
P = 128
IB = 1024            # i-block (columns of S^T per PSUM accumulation round)
NIB = N // IB        # 4
NJC = N // P         # 32 j-chunks
NPAIR = NJC // 2     # 16 DoubleRow pair-chunks
FD = 512             # matmul free-dim chunk (one PSUM bank of fp32)
NCORES = 8

# Schraudolph constants for e5m2: bits = rint(C0*s + C1)
C0 = 4.0 / np.log(2.0)
C1 = 59.8

_CACHE = {}


def _build():
    key = "nc"
    if key in _CACHE:
        return _CACHE[key]
    from contextlib import ExitStack
    import concourse.bacc as bacc
    import concourse.tile as tile
    from concourse import mybir

    f32 = mybir.dt.float32
    bf16 = mybir.dt.bfloat16
    e4 = mybir.dt.float8e4
    e5 = mybir.dt.float8e5
    i8 = mybir.dt.int8
    EXP = mybir.ActivationFunctionType.Exp
    DR = mybir.MatmulPerfMode.DoubleRow

    nc = bacc.Bacc("TRN2", target_bir_lowering=False, debug=False, num_devices=NCORES)

    x8_d = nc.dram_tensor("x8", [P, 2, N], e4, kind="ExternalInput").ap()
    xg_d = nc.dram_tensor("xg", [2, P, N], f32, kind="ExternalInput").ap()
    wq8_d = nc.dram_tensor("wq8", [P, 2, CI], e4, kind="ExternalInput").ap()
    wk8_d = nc.dram_tensor("wk8", [P, 2, CI], e4, kind="ExternalInput").ap()
    wv8_d = nc.dram_tensor("wv8", [P, 2, CI], e4, kind="ExternalInput").ap()
    wog_d = nc.dram_tensor("wog", [P, C], bf16, kind="ExternalInput").ap()
    bq_d = nc.dram_tensor("bq", [P, 1], f32, kind="ExternalInput").ap()
    bk_d = nc.dram_tensor("bk", [P, 1], f32, kind="ExternalInput").ap()
    out_d = nc.dram_tensor("out", [C, N], f32, kind="ExternalOutput").ap()

    with tile.TileContext(nc) as tc, ExitStack() as ctx:
        sb = ctx.enter_context(tc.tile_pool(name="sb", bufs=1))
        wk_pool = ctx.enter_context(tc.tile_pool(name="wk", bufs=1))
        ps = ctx.enter_context(tc.tile_pool(name="ps", bufs=1, space="PSUM"))

        # ---- persistent SBUF ----
        x8_s = sb.tile([P, 2, N], e4, name="x8_s")
        Xg = [sb.tile([P, N], f32, name=f"Xg{c}") for c in range(2)]
        Qp = sb.tile([P, 2, N], e4, name="Qp")      # [:,1,:] zeroed
        Kp = sb.tile([P, 2, N], e4, name="Kp")
        VT = sb.tile([P, NJC, P], e4, name="VT")    # [j-in-chunk, jc, ci]
        wq8_s = sb.tile([P, 2, CI], e4, name="wq8_s")
        wk8_s = sb.tile([P, 2, CI], e4, name="wk8_s")
        wv8_s = sb.tile([P, 2, CI], e4, name="wv8_s")
        wog_s = sb.tile([P, C], bf16, name="wog_s")
        bq_s = sb.tile([P, 1], f32, name="bq_s")
        bk_s = sb.tile([P, 1], f32, name="bk_s")
        ones_s = sb.tile([P, 2, P], e4, name="ones_s")

        # ---- input DMAs (weights first, then x8, xg in background) ----
        nc.sync.dma_start(out=wq8_s, in_=wq8_d)
        nc.sync.dma_start(out=wk8_s, in_=wk8_d)
        nc.sync.dma_start(out=wv8_s, in_=wv8_d)
        nc.sync.dma_start(out=wog_s, in_=wog_d)
        nc.sync.dma_start(out=bq_s, in_=bq_d)
        nc.sync.dma_start(out=bk_s, in_=bk_d)
        for q in range(4):
            sl = slice(q * IB, (q + 1) * IB)
            nc.sync.dma_start(out=x8_s[:, :, sl], in_=x8_d[:, :, sl])
        for cc in range(2):
            for q in range(2):
                sl = slice(q * 2048, (q + 1) * 2048)
                nc.sync.dma_start(out=Xg[cc][:, sl], in_=xg_d[cc, :, sl])

        nc.vector.memset(ones_s, 1.0)
        nc.gpsimd.memset(Qp[:, 1, :], 0.0)
        nc.gpsimd.memset(Kp[:, 1, :], 0.0)

        # ---- Q/K/V projections (DoubleRow over C=256), interleaved per slice ----
        for s8 in range(4):
            sl = slice(s8 * IB, (s8 + 1) * IB)
            pq = ps.tile([P, IB], f32, tag="st", bufs=2, name=f"pq{s8}")
            for h in range(2):
                hs = slice(s8 * IB + h * FD, s8 * IB + (h + 1) * FD)
                nc.tensor.matmul(pq[:, h * FD:(h + 1) * FD], lhsT=wq8_s,
                                 rhs=x8_s[:, :, hs], start=True, stop=True,
                                 perf_mode=DR)
            nc.vector.tensor_scalar_add(out=Qp[:, 0, sl], in0=pq, scalar1=bq_s)

            pk = ps.tile([P, IB], f32, tag="st", bufs=2, name=f"pk{s8}")
            for h in range(2):
                hs = slice(s8 * IB + h * FD, s8 * IB + (h + 1) * FD)
                nc.tensor.matmul(pk[:, h * FD:(h + 1) * FD], lhsT=wk8_s,
                                 rhs=x8_s[:, :, hs], start=True, stop=True,
                                 perf_mode=DR)
            nc.vector.tensor_scalar_add(out=Kp[:, 0, sl], in0=pk, scalar1=bk_s)

            pv = ps.tile([P, IB], f32, tag="st", bufs=2, name=f"pv{s8}")
            for c8 in range(8):
                jc = s8 * 8 + c8
                jsl = slice(jc * P, (jc + 1) * P)
                nc.tensor.matmul(pv[:, c8 * P:(c8 + 1) * P],
                                 lhsT=x8_s[:, :, jsl], rhs=wv8_s,
                                 start=True, stop=True, perf_mode=DR)
            nc.vector.tensor_copy(out=VT[:, s8 * 8:(s8 + 1) * 8, :], in_=pv)

        # ---- attention main loop ----
        def do_st(ib, jc, a_out):
            """S^T chunk [j=128, i=IB] -> exp -> e5m2 into a_out ([128, IB])."""
            i0 = ib * IB
            st_ps = ps.tile([P, IB], f32, tag="st", bufs=2, name=f"st{ib}_{jc}")
            for h in range(2):
                nc.tensor.matmul(
                    st_ps[:, h * FD:(h + 1) * FD],
                    lhsT=Kp[:, :, jc * P:(jc + 1) * P],
                    rhs=Qp[:, :, i0 + h * FD: i0 + (h + 1) * FD],
                    start=True, stop=True, perf_mode=DR)
            # per-ib pattern: pair parity 1 of most pairs goes to DVE
            on_dve = (jc % 2 == 1) and (jc % 8 != 7)
            if on_dve:
                nc.vector.tensor_scalar(
                    out=a_out.bitcast(i8), in0=st_ps,
                    scalar1=float(C0), scalar2=float(C1),
                    op0=mybir.AluOpType.mult, op1=mybir.AluOpType.add)
            else:
                nc.scalar.activation(a_out, st_ps, EXP)

        for ib in range(NIB):
            i0 = ib * IB
            o_ps = ps.tile([P, IB], f32, tag="o", bufs=1, name=f"o{ib}")
            s_ps = ps.tile([P, IB], f32, tag="sums", bufs=1, name=f"s{ib}")
            for t in range(NPAIR):
                a_pair = wk_pool.tile([P, 2, IB], e5, tag="a", bufs=4,
                                      name=f"a{ib}_{t}")
                do_st(ib, 2 * t, a_pair[:, 0, :])
                do_st(ib, 2 * t + 1, a_pair[:, 1, :])
                for h in range(2):
                    hs = slice(h * FD, (h + 1) * FD)
                    nc.tensor.matmul(
                        o_ps[:, hs], lhsT=VT[:, 2 * t:2 * t + 2, :],
                        rhs=a_pair[:, :, hs],
                        start=(t == 0), stop=(t == NPAIR - 1), perf_mode=DR)
                    nc.tensor.matmul(
                        s_ps[:, hs], lhsT=ones_s, rhs=a_pair[:, :, hs],
                        start=(t == 0), stop=(t == NPAIR - 1), perf_mode=DR)

            rec = wk_pool.tile([P, IB], f32, tag="rec", bufs=2, name=f"rec{ib}")
            nc.vector.reciprocal(rec, s_ps)
            onorm = wk_pool.tile([P, IB], bf16, tag="onorm", bufs=2,
                                 name=f"on{ib}")
            nc.vector.tensor_mul(onorm, o_ps, rec)

            # ---- output projection + residual for this i-block ----
            for ch in range(2):
                z_ps = ps.tile([P, IB], f32, tag="st", bufs=2,
                               name=f"z{ib}_{ch}")
                for h in range(2):
                    hs = slice(h * FD, (h + 1) * FD)
                    nc.tensor.matmul(
                        z_ps[:, hs], lhsT=wog_s[:, ch * CI:(ch + 1) * CI],
                        rhs=onorm[:, hs], start=True, stop=True)
                y_sb = wk_pool.tile([P, IB], f32, tag="y", bufs=2,
                                    name=f"y{ib}_{ch}")
                nc.vector.tensor_add(y_sb, z_ps, Xg[ch][:, i0:i0 + IB])
                nc.sync.dma_start(
                    out=out_d[ch * P:(ch + 1) * P, i0:i0 + IB], in_=y_sb)

    nc.compile()
    _CACHE[key] = nc
    return nc


def _in_maps(x, wq, bq, wk, bk, wv, bv, wo, bo, gamma):
    e4 = ml_dtypes.float8_e4m3
    bf = ml_dtypes.bfloat16
    x = np.asarray(x, np.float32).reshape(B, C, N)
    wq = np.asarray(wq, np.float32)
    wk = np.asarray(wk, np.float32)
    wv = np.asarray(wv, np.float32)
    wo = np.asarray(wo, np.float32)
    bq = np.asarray(bq, np.float32)
    bk = np.asarray(bk, np.float32)
    bv = np.asarray(bv, np.float32)
    bo = np.asarray(bo, np.float32)
    g = float(np.asarray(gamma, np.float32)[0])

    def wprep(w):  # [CI, C] -> [p, t, m] = w[m, t*128+p]
        return np.ascontiguousarray(
            w.T.reshape(2, P, CI).transpose(1, 0, 2)).astype(e4)

    wq8, wk8, wv8 = wprep(wq), wprep(wk), wprep(wv)
    wog = np.ascontiguousarray((g * wo).T).astype(bf)          # [CI, C]
    gadd = (g * (wo @ bv + bo)).astype(np.float32)             # [C]
    bq2 = np.ascontiguousarray(bq.reshape(P, 1))
    bk2 = np.ascontiguousarray(bk.reshape(P, 1))

    maps = []
    for b in range(B):
        xb = x[b]                                              # [C, N]
        x8 = np.ascontiguousarray(
            xb.reshape(2, P, N).transpose(1, 0, 2)).astype(e4)
        xg = np.ascontiguousarray(
            (xb + gadd[:, None]).reshape(2, P, N))
        maps.append(dict(
            x8=x8, xg=xg, wq8=wq8, wk8=wk8, wv8=wv8, wog=wog,
            bq=bq2, bk=bk2))
    return maps


def run(trace=False, **inputs):
    import concourse.bass_utils as bass_utils
    nc = _build()
    maps = _in_maps(**inputs)
    res = bass_utils.run_bass_kernel_spmd(
        nc, maps, core_ids=list(range(NCORES)), trace=trace)
    out = np.stack([r["out"] for r in res.results])
    return out.reshape(B, C, HH, WW).astype(np.float32), res


def kernel(**inputs):
    out, _ = run(trace=False, **inputs)
    return out


# revision 24
# speedup vs baseline: 1.8933x; 1.5446x over previous
"""NonLocalAttention Trainium2 kernel (fp8 DoubleRow + split-engine softmax).

Math per batch b (reference):
  q/k/v = conv1x1(x, w*, b*)            # [CI, N], N = H*W = 4096, CI = 128
  attn  = softmax(q^T k, axis=-1)       # [N, N]
  o     = v @ attn^T                    # [CI, N]
  out   = gamma * (wo @ o + bo) + x     # [C, N]

Distribution: data-parallel over batch, one batch per NeuronCore (B = 8).

Per-core design:
  - All heavy matmuls run in fp8 with MatmulPerfMode.DoubleRow (2 k-subtiles of
    128 contracted per pass at 0.5 cyc/row):
      * Q/K/V projections contract C=256 -> full-rate DoubleRow.
      * S^T = K^T Q contracts CI=128 -> DoubleRow with a zeroed 2nd k-subtile.
      * O = V @ A and sums = 1^T A contract N=4096 -> 16 pair-chunks each.
  - softmax (no max subtraction needed; |logits| <= ~9):
      * A = exp(S^T) stored as fp8e5m2 (max 57344 > e^9; sums are computed from
        the SAME quantized A so the normalization stays consistent).
      * exp work is split chunk-wise across ScalarE (native Exp) and DVE
        (Schraudolph bit trick: e5m2 bits of exp(s) == rint(4/ln2 * s + c),
        via tensor_scalar into an int8 alias of the fp8 tile).
  - Softmax row-normalization commutes with the V and wo matmuls; the V-bias
    contributes bv exactly after normalization, so host folds gamma*(wo@bv+bo)
    into the residual tensor xg = x + gamma*(wo@bv + bo) and gamma into
    wog = gamma*wo^T. Final: out = wog^T @ (O/sums) + xg.
  - Pipeline: the exp chain (PE S^T -> ScalarE/DVE exp) is the bottleneck, so
    it gets 3 rotating PSUM tiles (6 banks) to hide the produce/consume
    semaphore turnaround. The whole i-block's A stays resident in SBUF
    (2 ping-pong tensors), so O/sums accumulation (half-width [128,512] PSUM
    tiles, 1 bank each) is decoupled and drains during the NEXT i-block's
    exp stream, as do the softmax tails.
"""

import numpy as np
import ml_dtypes

B, C = 8, 256
HH, WW = 64, 64
N = HH * WW          # 4096
CI = 128
P = 128
IB = 1024            # i-block (columns of S^T per exp round)
NIB = N // IB        # 4
NJC = N // P         # 32 j-chunks
NPAIR = NJC // 2     # 16 DoubleRow pair-chunks per i-block
FD = 512             # matmul free-dim chunk (one PSUM bank of fp32)
NCORES = 8

# Schraudolph constants for e5m2: bits = rint(C0*s + C1)
C0 = 4.0 / np.log(2.0)
C1 = 59.8

# pairs whose second chunk ALSO goes to ScalarE (instead of DVE), per i-block —
# denser where DVE does filler work (projection casts, softmax tails)
AA_PAIRS = {
    0: (0, 2, 4, 6),
    1: (0, 4, 8, 12),
    2: (0, 4, 8, 12),
    3: (0, 4, 8, 12),
}

_CACHE = {}


def _build():
    key = "nc"
    if key in _CACHE:
        return _CACHE[key]
    from contextlib import ExitStack
    import concourse.bacc as bacc
    import concourse.tile as tile
    from concourse import mybir

    f32 = mybir.dt.float32
    bf16 = mybir.dt.bfloat16
    e4 = mybir.dt.float8e4
    e5 = mybir.dt.float8e5
    i8 = mybir.dt.int8
    EXP = mybir.ActivationFunctionType.Exp
    IDENT = mybir.ActivationFunctionType.Identity
    DR = mybir.MatmulPerfMode.DoubleRow

    nc = bacc.Bacc("TRN2", target_bir_lowering=False, debug=False, num_devices=NCORES)

    x8_d = nc.dram_tensor("x8", [P, 2, N], e4, kind="ExternalInput").ap()
    xg_d = nc.dram_tensor("xg", [2, P, N], f32, kind="ExternalInput").ap()
    wq8_d = nc.dram_tensor("wq8", [P, 2, CI], e4, kind="ExternalInput").ap()
    wk8_d = nc.dram_tensor("wk8", [P, 2, CI], e4, kind="ExternalInput").ap()
    wv8_d = nc.dram_tensor("wv8", [P, 2, CI], e4, kind="ExternalInput").ap()
    wog_d = nc.dram_tensor("wog", [P, C], bf16, kind="ExternalInput").ap()
    bq_d = nc.dram_tensor("bq", [P, 1], f32, kind="ExternalInput").ap()
    bk_d = nc.dram_tensor("bk", [P, 1], f32, kind="ExternalInput").ap()
    out_d = nc.dram_tensor("out", [C, N], f32, kind="ExternalOutput").ap()

    with tile.TileContext(nc) as tc, ExitStack() as ctx:
        sb = ctx.enter_context(tc.tile_pool(name="sb", bufs=1))
        wk_pool = ctx.enter_context(tc.tile_pool(name="wk", bufs=1))
        ps = ctx.enter_context(tc.tile_pool(name="ps", bufs=1, space="PSUM"))

        # ---- persistent SBUF ----
        x8_s = sb.tile([P, 2, N], e4, name="x8_s")
        Xg = [sb.tile([P, N], f32, name=f"Xg{c}") for c in range(2)]
        Qp = sb.tile([P, 2, N], e4, name="Qp")      # [:,1,:] zeroed
        Kp = sb.tile([P, 2, N], e4, name="Kp")
        VT = sb.tile([P, NJC, P], e4, name="VT")    # [j-in-chunk, jc, ci]
        Amat = [sb.tile([P, NJC, IB], e5, name=f"Amat{k}") for k in range(2)]
        wq8_s = sb.tile([P, 2, CI], e4, name="wq8_s")
        wk8_s = sb.tile([P, 2, CI], e4, name="wk8_s")
        wv8_s = sb.tile([P, 2, CI], e4, name="wv8_s")
        wog_s = sb.tile([P, C], bf16, name="wog_s")
        bq_s = sb.tile([P, 1], f32, name="bq_s")
        bk_s = sb.tile([P, 1], f32, name="bk_s")
        ones_s = sb.tile([P, 2, P], e4, name="ones_s")

        # ---- input DMAs (ordered so the first projections unblock earliest) ----
        for q2 in range(2):
            sl0 = slice(q2 * FD, (q2 + 1) * FD)
            nc.sync.dma_start(out=x8_s[:, :, sl0], in_=x8_d[:, :, sl0])
        nc.sync.dma_start(out=wq8_s, in_=wq8_d)
        nc.sync.dma_start(out=wk8_s, in_=wk8_d)
        nc.sync.dma_start(out=bq_s, in_=bq_d)
        nc.sync.dma_start(out=bk_s, in_=bk_d)
        for q in range(1, 4):
            sl = slice(q * IB, (q + 1) * IB)
            nc.sync.dma_start(out=x8_s[:, :, sl], in_=x8_d[:, :, sl])
        nc.sync.dma_start(out=wv8_s, in_=wv8_d)
        nc.sync.dma_start(out=wog_s, in_=wog_d)
        for cc in range(2):
            for q in range(2):
                sl = slice(q * 2048, (q + 1) * 2048)
                nc.sync.dma_start(out=Xg[cc][:, sl], in_=xg_d[cc, :, sl])

        nc.vector.memset(ones_s, 1.0)
        # zero k-subtile-1 pads of Q/K, sliced so early chunks unblock fast
        for q in range(4):
            sl = slice(q * IB, (q + 1) * IB)
            nc.gpsimd.memset(Qp[:, 1, sl], 0.0)
            nc.gpsimd.memset(Kp[:, 1, sl], 0.0)

        # ---- projection emitters (interleaved into the pair stream) ----
        def emit_q(s8, split=False):
            pq = ps.tile([P, IB], f32, tag="st", bufs=3, name=f"pq{s8}")
            for h in range(2):
                hs = slice(s8 * IB + h * FD, s8 * IB + (h + 1) * FD)
                nc.tensor.matmul(pq[:, h * FD:(h + 1) * FD], lhsT=wq8_s,
                                 rhs=x8_s[:, :, hs], start=True, stop=True,
                                 perf_mode=DR)
                if split:
                    nc.vector.tensor_scalar_add(
                        out=Qp[:, 0, hs], in0=pq[:, h * FD:(h + 1) * FD],
                        scalar1=bq_s)
            if not split:
                sl = slice(s8 * IB, (s8 + 1) * IB)
                nc.vector.tensor_scalar_add(out=Qp[:, 0, sl], in0=pq,
                                            scalar1=bq_s)

        def emit_k(s8, on_act=False):
            sl = slice(s8 * IB, (s8 + 1) * IB)
            pk = ps.tile([P, IB], f32, tag="st", bufs=3, name=f"pk{s8}")
            for h in range(2):
                hs = slice(s8 * IB + h * FD, s8 * IB + (h + 1) * FD)
                nc.tensor.matmul(pk[:, h * FD:(h + 1) * FD], lhsT=wk8_s,
                                 rhs=x8_s[:, :, hs], start=True, stop=True,
                                 perf_mode=DR)
                if on_act:
                    nc.scalar.activation(Kp[:, 0, hs],
                                         pk[:, h * FD:(h + 1) * FD],
                                         IDENT, bias=bk_s)
            if not on_act:
                nc.vector.tensor_scalar_add(out=Kp[:, 0, sl], in0=pk,
                                            scalar1=bk_s)

        def emit_v(s8):
            pv = ps.tile([P, IB], f32, tag="st", bufs=3, name=f"pv{s8}")
            for c8 in range(8):
                jc = s8 * 8 + c8
                jsl = slice(jc * P, (jc + 1) * P)
                nc.tensor.matmul(pv[:, c8 * P:(c8 + 1) * P],
                                 lhsT=x8_s[:, :, jsl], rhs=wv8_s,
                                 start=True, stop=True, perf_mode=DR)
            nc.vector.tensor_copy(out=VT[:, s8 * 8:(s8 + 1) * 8, :], in_=pv)

        # ---- exp stream ----
        def emit_st_exp(ib, t, parity):
            i0 = ib * IB
            jc = 2 * t + parity
            A = Amat[ib % 2]
            a_out = A[:, jc, :]
            st_ps = ps.tile([P, IB], f32, tag="st", bufs=3,
                            name=f"st{ib}_{jc}")
            for h in range(2):
                nc.tensor.matmul(
                    st_ps[:, h * FD:(h + 1) * FD],
                    lhsT=Kp[:, :, jc * P:(jc + 1) * P],
                    rhs=Qp[:, :, i0 + h * FD: i0 + (h + 1) * FD],
                    start=True, stop=True, perf_mode=DR)
            on_dve = (parity == 1) and (t not in AA_PAIRS[ib])
            if on_dve:
                nc.vector.tensor_scalar(
                    out=a_out.bitcast(i8), in0=st_ps,
                    scalar1=float(C0), scalar2=float(C1),
                    op0=mybir.AluOpType.mult, op1=mybir.AluOpType.add)
            else:
                nc.scalar.activation(a_out, st_ps, EXP)

        # ---- deferred O/sums/tail work (drains during the next i-block) ----
        def make_work(ib):
            """Deferred O/sums/tail work items for i-block ib."""
            A = Amat[ib % 2]
            i0 = ib * IB
            state = {}
            # the final i-block's half-B accumulates in the st pool (idle by
            # then), so its matmuls can overlap the half-A tail
            st_accum = ib == NIB - 1

            def mk_osums(u, t):
                def f():
                    if t == 0:
                        if u == 1 and st_accum:
                            state["o1"] = ps.tile([P, FD], f32, tag="st",
                                                  bufs=3, name=f"o{ib}_1")
                            state["s1"] = ps.tile([P, FD], f32, tag="st",
                                                  bufs=3, name=f"s{ib}_1")
                        else:
                            state[f"o{u}"] = ps.tile([P, FD], f32, tag="o",
                                                     bufs=1, name=f"o{ib}_{u}")
                            state[f"s{u}"] = ps.tile([P, FD], f32, tag="sums",
                                                     bufs=1, name=f"s{ib}_{u}")
                    hs = slice(u * FD, (u + 1) * FD)
                    nc.tensor.matmul(
                        state[f"o{u}"], lhsT=VT[:, 2 * t:2 * t + 2, :],
                        rhs=A[:, 2 * t:2 * t + 2, hs],
                        start=(t == 0), stop=(t == NPAIR - 1), perf_mode=DR)
                    nc.tensor.matmul(
                        state[f"s{u}"], lhsT=ones_s,
                        rhs=A[:, 2 * t:2 * t + 2, hs],
                        start=(t == 0), stop=(t == NPAIR - 1), perf_mode=DR)
                return f

            def mk_recnorm(u):
                def f():
                    rec = wk_pool.tile([P, FD], f32, tag="rec", bufs=2,
                                       name=f"rec{ib}_{u}")
                    nc.vector.reciprocal(rec, state[f"s{u}"])
                    onorm = wk_pool.tile([P, FD], bf16, tag="onorm", bufs=2,
                                         name=f"on{ib}_{u}")
                    nc.vector.tensor_mul(onorm, state[f"o{u}"], rec)
                    state[f"on{u}"] = onorm
                return f

            def mk_zy(u, ch):
                def f():
                    z = ps.tile([P, FD], f32, tag="st", bufs=3,
                                name=f"z{ib}_{u}_{ch}")
                    nc.tensor.matmul(
                        z, lhsT=wog_s[:, ch * CI:(ch + 1) * CI],
                        rhs=state[f"on{u}"], start=True, stop=True)
                    y = wk_pool.tile([P, FD], f32, tag="y", bufs=3,
                                     name=f"y{ib}_{u}_{ch}")
                    nc.vector.tensor_add(
                        y, z, Xg[ch][:, i0 + u * FD:i0 + (u + 1) * FD])
                    nc.sync.dma_start(
                        out=out_d[ch * P:(ch + 1) * P,
                                  i0 + u * FD:i0 + (u + 1) * FD], in_=y)
                return f

            oa = [mk_osums(0, t) for t in range(NPAIR)]
            if st_accum:
                rest = [mk_osums(1, t) for t in range(NPAIR)]
                rest += [mk_recnorm(0), mk_recnorm(1),
                         mk_zy(1, 0), mk_zy(0, 0), mk_zy(1, 1), mk_zy(0, 1)]
            else:
                rest = [mk_recnorm(0), mk_zy(0, 0), mk_zy(0, 1)]
                rest += [mk_osums(1, t) for t in range(NPAIR)]
                rest += [mk_recnorm(1), mk_zy(1, 0), mk_zy(1, 1)]
            return oa, rest

        # ---- software-pipelined emission ----
        emit_q(0, split=True)
        emit_k(0, on_act=True)

        work = []                               # deferred items, FIFO
        last_work = None                        # (oa, rest) of the final ib
        for g in range(NIB * NPAIR):
            ib, t = g // NPAIR, g % NPAIR
            if ib == 0:
                if t in (0, 4, 8, 12):
                    emit_v(t // 4)
                if t in (2, 6, 10):
                    emit_k(t // 4 + 1)
            if t == 12 and ib + 1 < NIB:
                emit_q(ib + 1)
            emit_st_exp(ib, t, 0)
            for _ in range(2):
                if work:
                    work.pop(0)()
            emit_st_exp(ib, t, 1)
            for _ in range(2):
                if work:
                    work.pop(0)()
            if ib == NIB - 1:
                # the final i-block's half-A accumulation drains inline
                # (the o/sums buffers free up a few pairs into this block)
                if last_work is None:
                    last_work = make_work(ib)
                if t == 6:
                    work.extend(last_work[0][:7])
                elif t > 6:
                    work.append(last_work[0][t])
                if t == NPAIR - 1:
                    work.extend(last_work[1])
            elif t == NPAIR - 1:
                oa, rest = make_work(ib)
                work.extend(oa)
                work.extend(rest)

        for f in work:
            f()

    nc.compile()
    _CACHE[key] = nc
    return nc


def _in_maps(x, wq, bq, wk, bk, wv, bv, wo, bo, gamma):
    e4 = ml_dtypes.float8_e4m3
    bf = ml_dtypes.bfloat16
    x = np.asarray(x, np.float32).reshape(B, C, N)
    wq = np.asarray(wq, np.float32)
    wk = np.asarray(wk, np.float32)
    wv = np.asarray(wv, np.float32)
    wo = np.asarray(wo, np.float32)
    bq = np.asarray(bq, np.float32)
    bk = np.asarray(bk, np.float32)
    bv = np.asarray(bv, np.float32)
    bo = np.asarray(bo, np.float32)
    g = float(np.asarray(gamma, np.float32)[0])

    def wprep(w):  # [CI, C] -> [p, t, m] = w[m, t*128+p]
        return np.ascontiguousarray(
            w.T.reshape(2, P, CI).transpose(1, 0, 2)).astype(e4)

    wq8, wk8, wv8 = wprep(wq), wprep(wk), wprep(wv)
    wog = np.ascontiguousarray((g * wo).T).astype(bf)          # [CI, C]
    gadd = (g * (wo @ bv + bo)).astype(np.float32)             # [C]
    bq2 = np.ascontiguousarray(bq.reshape(P, 1))
    bk2 = np.ascontiguousarray(bk.reshape(P, 1))

    maps = []
    for b in range(B):
        xb = x[b]                                              # [C, N]
        x8 = np.ascontiguousarray(
            xb.reshape(2, P, N).transpose(1, 0, 2)).astype(e4)
        xg = np.ascontiguousarray(
            (xb + gadd[:, None]).reshape(2, P, N))
        maps.append(dict(
            x8=x8, xg=xg, wq8=wq8, wk8=wk8, wv8=wv8, wog=wog,
            bq=bq2, bk=bk2))
    return maps


def run(trace=False, **inputs):
    import concourse.bass_utils as bass_utils
    nc = _build()
    maps = _in_maps(**inputs)
    res = bass_utils.run_bass_kernel_spmd(
        nc, maps, core_ids=list(range(NCORES)), trace=trace)
    out = np.stack([r["out"] for r in res.results])
    return out.reshape(B, C, HH, WW).astype(np.float32), res


def kernel(**inputs):
    out, _ = run(trace=False, **inputs)
    return out


# revision 38
# speedup vs baseline: 1.9376x; 1.0234x over previous
"""NonLocalAttention Trainium2 kernel (fp8 DoubleRow + split-engine softmax).

Math per batch b (reference):
  q/k/v = conv1x1(x, w*, b*)            # [CI, N], N = H*W = 4096, CI = 128
  attn  = softmax(q^T k, axis=-1)       # [N, N]
  o     = v @ attn^T                    # [CI, N]
  out   = gamma * (wo @ o + bo) + x     # [C, N]

Distribution: data-parallel over batch, one batch per NeuronCore (B = 8).

Per-core design:
  - All heavy matmuls run in fp8 with MatmulPerfMode.DoubleRow (2 k-subtiles of
    128 contracted per pass at 0.5 cyc/row):
      * Q/K/V projections contract C=256 -> full-rate DoubleRow.
      * S^T = K^T Q contracts CI=128 -> DoubleRow with a zeroed 2nd k-subtile.
      * O = V @ A and sums = 1^T A contract N=4096 -> 16 pair-chunks each.
  - softmax (no max subtraction needed; |logits| <= ~9):
      * A = exp(S^T) stored as fp8e5m2 (max 57344 > e^9; sums are computed from
        the SAME quantized A so the normalization stays consistent).
      * exp work is split chunk-wise across ScalarE (native Exp) and DVE
        (Schraudolph bit trick: e5m2 bits of exp(s) == rint(4/ln2 * s + c),
        via tensor_scalar into an int8 alias of the fp8 tile).
  - Softmax row-normalization commutes with the V and wo matmuls; the V-bias
    contributes bv exactly after normalization, so host folds gamma*(wo@bv+bo)
    into the residual tensor xg = x + gamma*(wo@bv + bo) and gamma into
    wog = gamma*wo^T. Final: out = wog^T @ (O/sums) + xg.
  - Pipeline: the exp chain (PE S^T -> ScalarE/DVE exp) is the bottleneck, so
    it gets 3 rotating PSUM tiles (6 banks) to hide the produce/consume
    semaphore turnaround. The whole i-block's A stays resident in SBUF
    (2 ping-pong tensors), so O/sums accumulation (half-width [128,512] PSUM
    tiles, 1 bank each) is decoupled and drains during the NEXT i-block's
    exp stream, as do the softmax tails.
"""

import numpy as np
import ml_dtypes

B, C = 8, 256
HH, WW = 64, 64
N = HH * WW          # 4096
CI = 128
P = 128
IB = 1024            # i-block (columns of S^T per exp round)
NIB = N // IB        # 4
NJC = N // P         # 32 j-chunks
NPAIR = NJC // 2     # 16 DoubleRow pair-chunks per i-block
FD = 512             # matmul free-dim chunk (one PSUM bank of fp32)
NCORES = 8

# Schraudolph constants for e5m2: bits = rint(C0*s + C1)
C0 = 4.0 / np.log(2.0)
C1 = 59.8

# pairs whose second chunk ALSO goes to ScalarE (instead of DVE), per i-block —
# denser where DVE does filler work (projection casts, softmax tails)
AA_PAIRS = {
    0: (0, 2, 4, 6),
    1: (0, 4, 8, 12),
    2: (0, 4, 8, 12),
    3: (0, 4, 8),
}

_CACHE = {}


def _build():
    key = "nc"
    if key in _CACHE:
        return _CACHE[key]
    from contextlib import ExitStack
    import concourse.bacc as bacc
    import concourse.tile as tile
    from concourse import mybir

    f32 = mybir.dt.float32
    bf16 = mybir.dt.bfloat16
    e4 = mybir.dt.float8e4
    e5 = mybir.dt.float8e5
    i8 = mybir.dt.int8
    EXP = mybir.ActivationFunctionType.Exp
    IDENT = mybir.ActivationFunctionType.Identity
    DR = mybir.MatmulPerfMode.DoubleRow

    nc = bacc.Bacc("TRN2", target_bir_lowering=False, debug=False, num_devices=NCORES)

    x8_d = nc.dram_tensor("x8", [P, 2, N], e4, kind="ExternalInput").ap()
    xg_d = nc.dram_tensor("xg", [2, P, N], f32, kind="ExternalInput").ap()
    # wkq8 packs [wk8 | wq8] along the last dim; bkq packs [bk | bq]
    wkq8_d = nc.dram_tensor("wkq8", [P, 2, 2 * CI], e4, kind="ExternalInput").ap()
    bkq_d = nc.dram_tensor("bkq", [P, 2], f32, kind="ExternalInput").ap()
    wv8_d = nc.dram_tensor("wv8", [P, 2, CI], e4, kind="ExternalInput").ap()
    wog_d = nc.dram_tensor("wog", [P, C], bf16, kind="ExternalInput").ap()
    out_d = nc.dram_tensor("out", [C, N], f32, kind="ExternalOutput").ap()

    with tile.TileContext(nc) as tc, ExitStack() as ctx:
        sb = ctx.enter_context(tc.tile_pool(name="sb", bufs=1))
        wk_pool = ctx.enter_context(tc.tile_pool(name="wk", bufs=1))
        ps = ctx.enter_context(tc.tile_pool(name="ps", bufs=1, space="PSUM"))

        # ---- persistent SBUF ----
        x8_s = sb.tile([P, 2, N], e4, name="x8_s")
        Xg = [sb.tile([P, N], f32, name=f"Xg{c}") for c in range(2)]
        Qp = sb.tile([P, 2, N], e4, name="Qp")      # [:,1,:] zeroed
        Kp = sb.tile([P, 2, N], e4, name="Kp")
        VT = sb.tile([P, NJC, P], e4, name="VT")    # [j-in-chunk, jc, ci]
        Amat = [sb.tile([P, NJC, IB], e5, name=f"Amat{k}") for k in range(2)]
        wkq8_s = sb.tile([P, 2, 2 * CI], e4, name="wkq8_s")
        wk8_s = wkq8_s[:, :, 0:CI]
        wq8_s = wkq8_s[:, :, CI:2 * CI]
        wv8_s = sb.tile([P, 2, CI], e4, name="wv8_s")
        wog_s = sb.tile([P, C], bf16, name="wog_s")
        bkq_s = sb.tile([P, 2], f32, name="bkq_s")
        bk_s = bkq_s[:, 0:1]
        bq_s = bkq_s[:, 1:2]
        ones_s = sb.tile([P, 2, P], e4, name="ones_s")

        # ---- input DMAs (ordered so the first projections unblock earliest) ----
        sl0 = slice(0, FD)
        nc.sync.dma_start(out=x8_s[:, :, sl0], in_=x8_d[:, :, sl0])
        nc.sync.dma_start(out=wkq8_s, in_=wkq8_d)
        nc.sync.dma_start(out=bkq_s, in_=bkq_d)
        sl0b = slice(FD, IB)
        nc.sync.dma_start(out=x8_s[:, :, sl0b], in_=x8_d[:, :, sl0b])
        for q in range(1, 4):
            sl = slice(q * IB, (q + 1) * IB)
            nc.sync.dma_start(out=x8_s[:, :, sl], in_=x8_d[:, :, sl])
        nc.sync.dma_start(out=wv8_s, in_=wv8_d)
        nc.sync.dma_start(out=wog_s, in_=wog_d)
        for cc in range(2):
            for q in range(2):
                sl = slice(q * 2048, (q + 1) * 2048)
                nc.sync.dma_start(out=Xg[cc][:, sl], in_=xg_d[cc, :, sl])

        nc.vector.memset(ones_s, 1.0)
        # zero k-subtile-1 pads of Q/K, sliced so early chunks unblock fast
        for q in range(4):
            sl = slice(q * IB, (q + 1) * IB)
            nc.gpsimd.memset(Qp[:, 1, sl], 0.0)
            nc.gpsimd.memset(Kp[:, 1, sl], 0.0)

        # ---- projection emitters (interleaved into the pair stream) ----
        def emit_q(s8, split=False):
            pq = ps.tile([P, IB], f32, tag="st", bufs=3, name=f"pq{s8}")
            for h in range(2):
                hs = slice(s8 * IB + h * FD, s8 * IB + (h + 1) * FD)
                nc.tensor.matmul(pq[:, h * FD:(h + 1) * FD], lhsT=wq8_s,
                                 rhs=x8_s[:, :, hs], start=True, stop=True,
                                 perf_mode=DR)
                if split:
                    nc.vector.tensor_scalar_add(
                        out=Qp[:, 0, hs], in0=pq[:, h * FD:(h + 1) * FD],
                        scalar1=bq_s)
            if not split:
                sl = slice(s8 * IB, (s8 + 1) * IB)
                nc.vector.tensor_scalar_add(out=Qp[:, 0, sl], in0=pq,
                                            scalar1=bq_s)

        def emit_k(s8, on_act=False):
            sl = slice(s8 * IB, (s8 + 1) * IB)
            pk = ps.tile([P, IB], f32, tag="st", bufs=3, name=f"pk{s8}")
            for h in range(2):
                hs = slice(s8 * IB + h * FD, s8 * IB + (h + 1) * FD)
                nc.tensor.matmul(pk[:, h * FD:(h + 1) * FD], lhsT=wk8_s,
                                 rhs=x8_s[:, :, hs], start=True, stop=True,
                                 perf_mode=DR)
                if on_act:
                    nc.scalar.activation(Kp[:, 0, hs],
                                         pk[:, h * FD:(h + 1) * FD],
                                         IDENT, bias=bk_s)
            if not on_act:
                nc.vector.tensor_scalar_add(out=Kp[:, 0, sl], in0=pk,
                                            scalar1=bk_s)

        def emit_v(s8):
            pv = ps.tile([P, IB], f32, tag="st", bufs=3, name=f"pv{s8}")
            for c8 in range(8):
                jc = s8 * 8 + c8
                jsl = slice(jc * P, (jc + 1) * P)
                nc.tensor.matmul(pv[:, c8 * P:(c8 + 1) * P],
                                 lhsT=x8_s[:, :, jsl], rhs=wv8_s,
                                 start=True, stop=True, perf_mode=DR)
            nc.vector.tensor_copy(out=VT[:, s8 * 8:(s8 + 1) * 8, :], in_=pv)

        # ---- exp stream ----
        def emit_st_exp(ib, t, parity):
            i0 = ib * IB
            jc = 2 * t + parity
            A = Amat[ib % 2]
            a_out = A[:, jc, :]
            st_ps = ps.tile([P, IB], f32, tag="st", bufs=3,
                            name=f"st{ib}_{jc}")
            for h in range(2):
                nc.tensor.matmul(
                    st_ps[:, h * FD:(h + 1) * FD],
                    lhsT=Kp[:, :, jc * P:(jc + 1) * P],
                    rhs=Qp[:, :, i0 + h * FD: i0 + (h + 1) * FD],
                    start=True, stop=True, perf_mode=DR)
            on_dve = (parity == 1) and (t not in AA_PAIRS[ib])
            if on_dve:
                nc.vector.tensor_scalar(
                    out=a_out.bitcast(i8), in0=st_ps,
                    scalar1=float(C0), scalar2=float(C1),
                    op0=mybir.AluOpType.mult, op1=mybir.AluOpType.add)
            else:
                nc.scalar.activation(a_out, st_ps, EXP)

        # ---- deferred O/sums/tail work (drains during the next i-block) ----
        def make_work(ib):
            """Deferred O/sums/tail work items for i-block ib."""
            A = Amat[ib % 2]
            i0 = ib * IB
            state = {}
            # the final i-block's half-B accumulates in the st pool (idle by
            # then), so its matmuls can overlap the half-A tail
            st_accum = ib == NIB - 1

            def mk_osums(u, t):
                def f():
                    if t == 0:
                        if u == 1 and st_accum:
                            state["o1"] = ps.tile([P, FD], f32, tag="st",
                                                  bufs=3, name=f"o{ib}_1")
                            state["s1"] = ps.tile([P, FD], f32, tag="st",
                                                  bufs=3, name=f"s{ib}_1")
                        else:
                            state[f"o{u}"] = ps.tile([P, FD], f32, tag="o",
                                                     bufs=1, name=f"o{ib}_{u}")
                            state[f"s{u}"] = ps.tile([P, FD], f32, tag="sums",
                                                     bufs=1, name=f"s{ib}_{u}")
                    hs = slice(u * FD, (u + 1) * FD)
                    nc.tensor.matmul(
                        state[f"o{u}"], lhsT=VT[:, 2 * t:2 * t + 2, :],
                        rhs=A[:, 2 * t:2 * t + 2, hs],
                        start=(t == 0), stop=(t == NPAIR - 1), perf_mode=DR)
                    nc.tensor.matmul(
                        state[f"s{u}"], lhsT=ones_s,
                        rhs=A[:, 2 * t:2 * t + 2, hs],
                        start=(t == 0), stop=(t == NPAIR - 1), perf_mode=DR)
                return f

            def mk_rec(u):
                def f():
                    state[f"rec{u}"] = wk_pool.tile([P, FD], f32, tag="rec",
                                                    bufs=2,
                                                    name=f"rec{ib}_{u}")
                    nc.vector.reciprocal(state[f"rec{u}"], state[f"s{u}"])
                return f

            def mk_onorm(u):
                def f():
                    onorm = wk_pool.tile([P, FD], bf16, tag="onorm", bufs=2,
                                         name=f"on{ib}_{u}")
                    nc.vector.tensor_mul(onorm, state[f"o{u}"],
                                         state[f"rec{u}"])
                    state[f"on{u}"] = onorm
                return f

            def mk_recnorm(u):
                def f():
                    mk_rec(u)()
                    mk_onorm(u)()
                return f

            def mk_zy(u, ch):
                def f():
                    z = ps.tile([P, FD], f32, tag="st", bufs=3,
                                name=f"z{ib}_{u}_{ch}")
                    nc.tensor.matmul(
                        z, lhsT=wog_s[:, ch * CI:(ch + 1) * CI],
                        rhs=state[f"on{u}"], start=True, stop=True)
                    y = wk_pool.tile([P, FD], f32, tag="y", bufs=3,
                                     name=f"y{ib}_{u}_{ch}")
                    nc.vector.tensor_add(
                        y, z, Xg[ch][:, i0 + u * FD:i0 + (u + 1) * FD])
                    nc.sync.dma_start(
                        out=out_d[ch * P:(ch + 1) * P,
                                  i0 + u * FD:i0 + (u + 1) * FD], in_=y)
                return f

            oa = [mk_osums(0, t) for t in range(NPAIR)]
            if st_accum:
                rest = [mk_osums(1, t) for t in range(NPAIR)]
                rest += [mk_rec(0), mk_rec(1), mk_onorm(0), mk_onorm(1),
                         mk_zy(0, 0), mk_zy(1, 0), mk_zy(0, 1), mk_zy(1, 1)]
            else:
                rest = [mk_recnorm(0), mk_zy(0, 0), mk_zy(0, 1)]
                rest += [mk_osums(1, t) for t in range(NPAIR)]
                rest += [mk_recnorm(1), mk_zy(1, 0), mk_zy(1, 1)]
            return oa, rest

        # ---- software-pipelined emission ----
        # first Q/K projections, per half, on the (still idle) o/sums PSUM
        # tags so the st pool starts unencumbered; order [K0h0,Q0h0,Q0h1,K0h1]
        for idx, (tg, w_s, b_s, OUT, hh) in enumerate((
                ("o", wk8_s, bk_s, Kp, 0), ("sums", wq8_s, bq_s, Qp, 0),
                ("sums", wq8_s, bq_s, Qp, 1), ("o", wk8_s, bk_s, Kp, 1))):
            hs = slice(hh * FD, (hh + 1) * FD)
            pt = ps.tile([P, FD], f32, tag=tg, bufs=1, name=f"pqk0_{idx}")
            nc.tensor.matmul(pt, lhsT=w_s, rhs=x8_s[:, :, hs],
                             start=True, stop=True, perf_mode=DR)
            nc.vector.tensor_scalar_add(out=OUT[:, 0, hs], in0=pt,
                                        scalar1=b_s)

        work = []                               # deferred items, FIFO
        last_work = None                        # (oa, rest) of the final ib
        for g in range(NIB * NPAIR):
            ib, t = g // NPAIR, g % NPAIR
            if ib == 0:
                if t in (1, 4, 8, 12):
                    emit_v(t // 4)
                if t in (2, 6, 10):
                    emit_k(t // 4 + 1)
            if t == 12 and ib + 1 < NIB:
                emit_q(ib + 1)
            emit_st_exp(ib, t, 0)
            for _ in range(2):
                if work:
                    work.pop(0)()
            emit_st_exp(ib, t, 1)
            for _ in range(2):
                if work:
                    work.pop(0)()
            if ib == NIB - 1:
                # the final i-block's half-A accumulation drains inline
                # (the o/sums buffers free up a few pairs into this block)
                if last_work is None:
                    last_work = make_work(ib)
                if t == 6:
                    work.extend(last_work[0][:7])
                elif t > 6:
                    work.append(last_work[0][t])
                if t == NPAIR - 1:
                    work.extend(last_work[1])
            elif t == NPAIR - 1:
                oa, rest = make_work(ib)
                work.extend(oa)
                work.extend(rest)

        for f in work:
            f()

    nc.compile()
    _CACHE[key] = nc
    return nc


def _in_maps(x, wq, bq, wk, bk, wv, bv, wo, bo, gamma):
    e4 = ml_dtypes.float8_e4m3
    bf = ml_dtypes.bfloat16
    x = np.asarray(x, np.float32).reshape(B, C, N)
    wq = np.asarray(wq, np.float32)
    wk = np.asarray(wk, np.float32)
    wv = np.asarray(wv, np.float32)
    wo = np.asarray(wo, np.float32)
    bq = np.asarray(bq, np.float32)
    bk = np.asarray(bk, np.float32)
    bv = np.asarray(bv, np.float32)
    bo = np.asarray(bo, np.float32)
    g = float(np.asarray(gamma, np.float32)[0])

    def wprep(w):  # [CI, C] -> [p, t, m] = w[m, t*128+p]
        return np.ascontiguousarray(
            w.T.reshape(2, P, CI).transpose(1, 0, 2)).astype(e4)

    wq8, wk8, wv8 = wprep(wq), wprep(wk), wprep(wv)
    wkq8 = np.ascontiguousarray(np.concatenate([wk8, wq8], axis=2))
    wog = np.ascontiguousarray((g * wo).T).astype(bf)          # [CI, C]
    gadd = (g * (wo @ bv + bo)).astype(np.float32)             # [C]
    bkq = np.ascontiguousarray(
        np.stack([bk, bq], axis=1).astype(np.float32))         # [P, 2]

    maps = []
    for b in range(B):
        xb = x[b]                                              # [C, N]
        x8 = np.ascontiguousarray(
            xb.reshape(2, P, N).transpose(1, 0, 2)).astype(e4)
        xg = np.ascontiguousarray(
            (xb + gadd[:, None]).reshape(2, P, N))
        maps.append(dict(
            x8=x8, xg=xg, wkq8=wkq8, wv8=wv8, wog=wog, bkq=bkq))
    return maps


def run(trace=False, **inputs):
    import concourse.bass_utils as bass_utils
    nc = _build()
    maps = _in_maps(**inputs)
    res = bass_utils.run_bass_kernel_spmd(
        nc, maps, core_ids=list(range(NCORES)), trace=trace)
    out = np.stack([r["out"] for r in res.results])
    return out.reshape(B, C, HH, WW).astype(np.float32), res


def kernel(**inputs):
    out, _ = run(trace=False, **inputs)
    return out
